# revision 22
# baseline (speedup 1.0000x reference)
"""Multi-head self-attention (B=16, N=1024, D=768, H=12) on 8 TRN2 NeuronCores.

Data-parallel over batch (2 batches per core, weights replicated, no
collectives). Per core, one fused Bass/Tile kernel:

  x --gpsimd casting-DMA (f32->f16, token-interleaved)--> x_h [128, 8*768]
      (token 8p+i on partition p, slot i; attention is permutation-invariant
      over tokens, so the interleave is only undone at the output DMA)
  x_h --XBAR DMA-transpose (8 instrs/batch)--> xT_b [128d, 6j, 1024tok]
  QT = (W_q^T x^T + b_q) in [col, tok] layout (f16).  KT = W_k^T x^T with NO
      bias: softmax is invariant to adding a per-query constant, and the
      k-bias contributes qT[n]*bk which is constant over keys -- exact drop.
  V_aug = [x W_v | ones-col per head]  [tok, 12*65] (f16)
  per head pair: S^T[m,n] = K Q^T (PE, row-group co-resident halves),
      E = exp(S^T*scale) (ACT, [128,1024]), O^T = V_aug^T E (PE; row 64 =
      softmax denominator via the ones column -- no max subtraction needed,
      scores are O(1)).  normalize: batched reciprocal over both heads
      [2,512], one DRAM store + one broadcast DMA, then DVE mul reading
      O^T straight out of PSUM.
  out = attnT^T W_proj + (W_proj^T b_v + b_proj)  (rank-1 bias matmul;
      V-bias folded through softmax since rows of A sum to 1)

All matmul operands f16 (1 cycle/row; fp32 runs 2-pass at 1/4 rate and
breaks HAM warm-up). PSUM accumulation is f32. Engine assignment: PE matmuls,
ACT exp only, DVE bias-adds/normalize, GPSIMD all psum->sbuf casts + W/x DMAs.
"""

import numpy as np

_CACHE: dict = {}

P = 128
BL, N, D, H, HD = 2, 1024, 768, 12, 64
D3 = 3 * D
SCALE = float(HD) ** -0.5


def _build():
    import concourse.mybir as mybir
    import concourse.tile as tile
    from concourse import bacc

    dt = mybir.dt
    F32, F16 = dt.float32, dt.float16
    AF = mybir.ActivationFunctionType

    nc = bacc.Bacc("TRN2", target_bir_lowering=False, debug=False)
    x_d = nc.dram_tensor("x", [BL, N, D], F32, kind="ExternalInput").ap()
    wqkv_d = nc.dram_tensor("W_qkv", [D, D3], F32, kind="ExternalInput").ap()
    bqkv_d = nc.dram_tensor("b_qkv", [D3], F32, kind="ExternalInput").ap()
    wproj_d = nc.dram_tensor("W_proj", [D, D], F32, kind="ExternalInput").ap()
    bproj_d = nc.dram_tensor("b_proj", [D], F32, kind="ExternalInput").ap()
    out_d = nc.dram_tensor("out", [BL, N, D], F32, kind="ExternalOutput").ap()
    # token-interleaved views: partition p, slot i <-> token 8p+i
    x_il = x_d.rearrange("b (p i) d -> b p (i d)", p=P)       # [2, 128, 6144]
    out_il = out_d.rearrange("b (p i) d -> b i p d", p=P)     # [2, 8, 128, 768]

    with tile.TileContext(nc) as tc:
        with tc.tile_pool(name="sb", bufs=1) as sb, \
             tc.tile_pool(name="dp", bufs=1, space="DRAM") as dp, \
             tc.tile_pool(name="ps", bufs=2, space="PSUM") as ps:

            # ---------- constants ----------
            ones_h = sb.tile([P, P], F16, tag="ones_h", bufs=1, name="ones_h")
            nc.vector.memset(ones_h[:], 1.0)

            # ---------- x + W loads: HWDGE f32 stage + gpsimd casts -------
            # x on sync queue; W staged on scalar queue (ACT idle pre-attn)
            x6 = {}
            x_h = {}
            x_h[0] = sb.tile([P, 8 * D], F16, tag="x_h", bufs=1, name="x_h")
            for t in range(8):
                xst = sb.tile([P, D], F32, tag="x6", bufs=4, name="xst")
                nc.sync.dma_start(xst[:], x_il[0][:, D * t:D * (t + 1)])
                nc.vector.tensor_copy(x_h[0][:, D * t:D * (t + 1)], xst[:])
            # separate Q/K/V weight tiles: a j-group read only deps on its
            # own block's single cast (dep tracking is conservative)
            wQ = [sb.tile([P, D], F16, tag=f"wq{d}", bufs=1, name=f"wq{d}")
                  for d in range(6)]
            wK = [sb.tile([P, D], F16, tag=f"wk{d}", bufs=1, name=f"wk{d}")
                  for d in range(6)]
            wV = [sb.tile([P, D], F16, tag=f"wv{d}", bufs=1, name=f"wv{d}")
                  for d in range(6)]

            def stage_w(dsttile, d, c0):
                wstg = sb.tile([P, D], F32, tag="wstage", bufs=3, name="wstg")
                nc.scalar.dma_start(
                    wstg[:], wqkv_d[P * d:P * (d + 1), c0:c0 + D])
                nc.vector.tensor_copy(dsttile[:], wstg[:])

            # wire order = need order: Q, K, V
            for d in range(6):
                stage_w(wQ[d], d, 0)
            for d in range(6):
                stage_w(wK[d], d, D)
            for d in range(6):
                stage_w(wV[d], d, 2 * D)

            # ---------- biases: pad-to-32 + XBAR transpose (sync q first)
            bstg = sb.tile([18, P], F32, tag="bstg", bufs=1, name="bstg")
            nc.sync.dma_start(bstg[0:18, :],
                              bqkv_d.rearrange("(j p) -> j p", p=P))
            bstg_h = sb.tile([32, P], F16, tag="bstg_h", bufs=1, name="bstg_h")
            nc.vector.memset(bstg_h[:], 0.0)
            nc.vector.tensor_copy(bstg_h[0:18, :], bstg[0:18, :])
            bqkvT_h = sb.tile([P, 32], F16, tag="bqkvT_h", bufs=1,
                              name="bqkvT_h")
            nc.sync.dma_start_transpose(bqkvT_h[:], bstg_h[:])
            bqkvT = sb.tile([P, 18], F32, tag="bqkvT", bufs=1, name="bqkvT")
            nc.vector.tensor_copy(bqkvT[:], bqkvT_h[:, 0:18])
            bv_h = bqkvT_h  # [:, 12:18] = V bias, f16, lhsT of bfinal matmul
            bprow_s = sb.tile([P, D], F32, tag="wstage", bufs=3, name="bprow_s")
            nc.scalar.dma_start(bprow_s[0:1, :], bproj_d.unsqueeze(0))
            bproj_row = sb.tile([1, D], F16, tag="bproj_row", bufs=1,
                                name="bproj_row")
            nc.vector.tensor_copy(bproj_row[:], bprow_s[0:1, :])

            # ---------- PE warm-up: ~4us of dummy matmuls flips HAM to 8/8
            warm_h = sb.tile([P, 512], F16, tag="e", bufs=3, name="warm_h")
            nc.vector.memset(warm_h[:], 0.0)
            for wi in range(28):
                wps = ps.tile([P, 512], F32, tag="mm", bufs=2, name="wps")
                nc.tensor.matmul(wps[:], ones_h[:, 0:P], warm_h[:],
                                 start=True, stop=True)

            # ---------- x transposes: XBAR DMA-transpose, no PE ----------
            # xT_b[b][dp, j, n] = x token (8c+i at n=128i+c), dim 128j+dp
            xT_b = {b: sb.tile([P, 6, N], F16, tag=f"xT{b}", bufs=1,
                               name=f"xT{b}") for b in range(BL)}
            for t in range(8):
                nc.sync.dma_start_transpose(
                    xT_b[0][:, :, P * t:P * (t + 1)],
                    x_h[0][:, D * t:D * (t + 1)])
            # x6/x_h buffers shared (bufs=1): b1 load after b0 readers
            x_h[1] = sb.tile([P, 8 * D], F16, tag="x_h", bufs=1, name="x_h")
            for t in range(8):
                xst = sb.tile([P, D], F32, tag="x6", bufs=4, name="xst")
                nc.sync.dma_start(xst[:], x_il[1][:, D * t:D * (t + 1)])
                nc.vector.tensor_copy(x_h[1][:, D * t:D * (t + 1)], xst[:])
            wp_h = [sb.tile([P, D], F16, tag=f"wproj{d}", bufs=1,
                            name=f"wproj{d}") for d in range(6)]
            for d in range(6):
                wstg = sb.tile([P, D], F32, tag="wstage", bufs=3, name="wstg")
                nc.scalar.dma_start(wstg[:], wproj_d[P * d:P * (d + 1), :])
                nc.vector.tensor_copy(wp_h[d][:], wstg[:])
            for t in range(8):
                nc.sync.dma_start_transpose(
                    xT_b[1][:, :, P * t:P * (t + 1)],
                    x_h[1][:, D * t:D * (t + 1)])

            # ---------- per-batch tiles ----------
            qk = {b: [sb.tile([P, N], F16, tag=f"qk{b % 2}_{j}", bufs=1,
                              name=f"qk{j}") for j in range(12)]
                  for b in range(BL)}
            v = {b: [sb.tile([P, 12 * 65], F16, tag=f"v{b % 2}_{t}", bufs=1,
                             name=f"v{t}") for t in range(8)]
                 for b in range(BL)}
            at = {b: [sb.tile([P, N], F16, tag=f"at{j}", bufs=1,
                              name=f"at{j}") for j in range(6)]
                  for b in range(BL)}

            def emit_qkv_group(b, j, nh):
                qps = ps.tile([P, 512], F32, tag="mm", bufs=2, name="qps")
                w, col = (wQ, j) if j < 6 else (wK, j - 6)
                for d in range(6):
                    nc.tensor.matmul(qps[:], w[d][:, P * col:P * (col + 1)],
                                     xT_b[b][:, d, 512 * nh:512 * (nh + 1)],
                                     start=(d == 0), stop=(d == 5))
                dst = qk[b][j][:, 512 * nh:512 * (nh + 1)]
                if j < 6:   # Q: add bias on DVE
                    nc.vector.tensor_scalar_add(dst, qps[:], bqkvT[:, j:j + 1])
                else:       # K: bias dropped (softmax-invariant); plain cast
                    nc.vector.tensor_copy(dst, qps[:])

            def emit_v_group(b, t, ci):
                c0, cw = ((0, 512), (512, 256))[ci]
                v3 = v[b][t].rearrange("p (h c) -> p h c", c=65)
                if ci == 0:
                    nc.vector.tensor_copy(v3[:, :, 64:65],
                                          ones_h[:, 0:12].unsqueeze(2))
                vps = ps.tile([P, 512], F32, tag="mm", bufs=2, name="vps")
                for d in range(6):
                    nc.tensor.matmul(vps[:, 0:cw],
                                     xT_b[b][:, d, P * t:P * (t + 1)],
                                     wV[d][:, c0:c0 + cw],
                                     start=(d == 0), stop=(d == 5))
                nc.vector.tensor_copy(
                    v3[:, (c0 // HD):((c0 + cw) // HD), 0:HD],
                    vps[:, 0:cw].rearrange("p (h c) -> p h c", c=HD))

            def emit_proj_half(b, t, ci):
                c0, cw = ((0, 512), (512, 256))[ci]
                pps = ps.tile([P, 512], F32, tag="mm", bufs=2, name="pps")
                for d in range(6):
                    nc.tensor.matmul(pps[:, 0:cw],
                                     at[b][d][:, P * t:P * (t + 1)],
                                     wp_h[d][:, c0:c0 + cw],
                                     start=(d == 0), stop=False)
                nc.tensor.matmul(pps[:, 0:cw], ones_h[0:1, 0:P],
                                 bfinal_h[:, c0:c0 + cw],
                                 start=False, stop=True)
                osb = sb.tile([P, 512], F32, tag="outs", bufs=1, name="osb")
                if b == 0:  # fillers inside attn-1: ACT is exp-saturated
                    nc.vector.tensor_copy(osb[:, 0:cw], pps[:, 0:cw])
                else:       # tail: ACT idle
                    nc.scalar.copy(osb[:, 0:cw], pps[:, 0:cw])
                nc.sync.dma_start(out_il[b, t][:, c0:c0 + cw], osb[:, 0:cw])

            def emit_proj_group(b, t):
                emit_proj_half(b, t, 0)
                emit_proj_half(b, t, 1)

            # b_final = W_proj^T b_v + b_proj   [1, 768] f16
            bfinal_h = sb.tile([1, D], F16, tag="bfinal", bufs=1, name="bfinal")

            def emit_bfinal():
                for c0, cw in ((0, 512), (512, 256)):
                    bf_ps = ps.tile([1, 512], F32, tag="mm", bufs=2,
                                    name="bf_ps")
                    for d in range(6):
                        nc.tensor.matmul(bf_ps[:, 0:cw],
                                         bv_h[:, 12 + d:13 + d],
                                         wp_h[d][:, c0:c0 + cw],
                                         start=(d == 0), stop=(d == 5))
                    nc.vector.tensor_add(bfinal_h[:, c0:c0 + cw],
                                         bf_ps[0:1, 0:cw],
                                         bproj_row[:, c0:c0 + cw])

            def emit_attention(b, fillers):
                for jp in range(6):
                    qt, kt = qk[b][jp], qk[b][6 + jp]
                    for nh in range(2):
                        n0 = 512 * nh
                        ot = [ps.tile([65, 512], F32, tag="ot", bufs=2,
                                      name="otps") for _ in range(2)]
                        pend = []
                        for m in range(8):
                            sps = ps.tile([P, N], F32, tag="s", bufs=2,
                                          name="sps")
                            for hh in range(2):
                                r0, r1 = HD * hh, HD * (hh + 1)
                                nc.tensor.matmul(
                                    sps[:, 512 * hh:512 * (hh + 1)],
                                    kt[r0:r1, P * m:P * (m + 1)],
                                    qt[r0:r1, n0:n0 + 512],
                                    start=True, stop=True)
                            e = sb.tile([P, N], F16, tag="e", bufs=3, name="e")
                            nc.scalar.activation(e[:], sps[:], AF.Exp,
                                                 scale=SCALE)
                            pend.append((m, e))
                            if len(pend) == 2:
                                pm, pe_ = pend.pop(0)
                                for hh in range(2):
                                    h = 2 * jp + hh
                                    nc.tensor.matmul(
                                        ot[hh][:],
                                        v[b][pm][:, 65 * h:65 * h + 65],
                                        pe_[:, 512 * hh:512 * (hh + 1)],
                                        start=(pm == 0), stop=(pm == 7))
                            if m in (1, 3, 5) and fillers:
                                fillers.pop(0)()
                        for pm, pe_ in pend:
                            for hh in range(2):
                                h = 2 * jp + hh
                                nc.tensor.matmul(
                                    ot[hh][:], v[b][pm][:, 65 * h:65 * h + 65],
                                    pe_[:, 512 * hh:512 * (hh + 1)],
                                    start=(pm == 0), stop=(pm == 7))
                        # normalize: copy O^T + denom out of PSUM early (frees
                        # ot banks), then batched recip/broadcast off-chain
                        u_sb = [sb.tile([HD, 512], F16, tag=f"u{hh}", bufs=1,
                                        name="u_sb") for hh in range(2)]
                        den = sb.tile([1, N], F32, tag="den", bufs=1,
                                      name="den")
                        nc.vector.tensor_copy(u_sb[0][:], ot[0][0:HD, :])
                        nc.vector.tensor_copy(den[:, 0:512], ot[0][64:65, :])
                        nc.vector.tensor_copy(u_sb[1][:], ot[1][0:HD, :])
                        nc.vector.tensor_copy(den[:, 512:1024],
                                              ot[1][64:65, :])
                        rr_f = sb.tile([1, N], F32, tag="rr_f", bufs=1,
                                       name="rr_f")
                        nc.vector.reciprocal_approx_fast(out=rr_f[:],
                                                         in_=den[:])
                        rr_h = sb.tile([1, N], F16, tag="rr_h", bufs=1,
                                       name="rr_h")
                        nc.vector.tensor_copy(rr_h[:], rr_f[:])
                        bc2 = sb.tile([HD, N], F16, tag="bc2", bufs=2,
                                      name="bc2")
                        nc.gpsimd.partition_broadcast(bc2[:], rr_h[:],
                                                      channels=HD)
                        for hh in range(2):
                            r0, r1 = HD * hh, HD * (hh + 1)
                            nc.vector.tensor_mul(
                                at[b][jp][r0:r1, n0:n0 + 512],
                                u_sb[hh][:],
                                bc2[:, 512 * hh:512 * (hh + 1)])
                        if fillers:
                            fillers.pop(0)()

            # ---------- schedule ----------
            # batch 0 QKV + V fully before attention (wire order matches);
            # PE stays continuously busy so HAM holds 8/8
            for j in range(12):
                for nh in range(2):
                    emit_qkv_group(0, j, nh)
            for t in range(8):
                for ci in range(2):
                    emit_v_group(0, t, ci)

            fill0 = [lambda j=j, nh=nh: emit_qkv_group(1, j, nh)
                     for jp_ in range(6) for j in (jp_, 6 + jp_)
                     for nh in range(2)]
            fill0 += [lambda t=t, ci=ci: emit_v_group(1, t, ci)
                      for t in range(8) for ci in range(2)]
            emit_attention(0, fill0)
            for f in fill0:
                f()

            # batch-1 attention with bfinal + batch-0 projection interleaved
            fill1 = [emit_bfinal]
            fill1 += [lambda t=t, ci=ci: emit_proj_half(0, t, ci)
                      for t in range(8) for ci in range(2)]
            emit_attention(1, fill1)
            for f in fill1:
                f()

            # batch-1 projection (tail)
            for t in range(8):
                emit_proj_group(1, t)
    nc.compile()
    return nc


def _get_nc():
    if "nc" not in _CACHE:
        _CACHE["nc"] = _build()
    return _CACHE["nc"]


def kernel(x, W_qkv, b_qkv, W_proj, b_proj):
    from concourse.bass_utils import run_bass_kernel_spmd

    nc = _get_nc()
    x = np.ascontiguousarray(x, dtype=np.float32)
    in_maps = [
        {
            "x": x[2 * i:2 * i + 2],
            "W_qkv": np.asarray(W_qkv, dtype=np.float32),
            "b_qkv": np.asarray(b_qkv, dtype=np.float32),
            "W_proj": np.asarray(W_proj, dtype=np.float32),
            "b_proj": np.asarray(b_proj, dtype=np.float32),
        }
        for i in range(8)
    ]
    res = run_bass_kernel_spmd(nc, in_maps, core_ids=list(range(8)))
    return np.concatenate([r["out"] for r in res.results], axis=0)


# revision 23
# speedup vs baseline: 1.0328x; 1.0328x over previous
"""Multi-head self-attention (B=16, N=1024, D=768, H=12) on 8 TRN2 NeuronCores.

Data-parallel over batch (2 batches per core, weights replicated, no
collectives). Per core, one fused Bass/Tile kernel:

  x --gpsimd casting-DMA (f32->f16, token-interleaved)--> x_h [128, 8*768]
      (token 8p+i on partition p, slot i; attention is permutation-invariant
      over tokens, so the interleave is only undone at the output DMA)
  x_h --XBAR DMA-transpose (8 instrs/batch)--> xT_b [128d, 6j, 1024tok]
  QT = (W_q^T x^T + b_q) in [col, tok] layout (f16).  KT = W_k^T x^T with NO
      bias: softmax is invariant to adding a per-query constant, and the
      k-bias contributes qT[n]*bk which is constant over keys -- exact drop.
  V_aug = [x W_v | ones-col per head]  [tok, 12*65] (f16)
  per head pair: S^T[m,n] = K Q^T (PE, row-group co-resident halves),
      E = exp(S^T*scale) (ACT, [128,1024]), O^T = V_aug^T E (PE; row 64 =
      softmax denominator via the ones column -- no max subtraction needed,
      scores are O(1)).  normalize: batched reciprocal over both heads
      [2,512], one DRAM store + one broadcast DMA, then DVE mul reading
      O^T straight out of PSUM.
  out = attnT^T W_proj + (W_proj^T b_v + b_proj)  (rank-1 bias matmul;
      V-bias folded through softmax since rows of A sum to 1)

All matmul operands f16 (1 cycle/row; fp32 runs 2-pass at 1/4 rate and
breaks HAM warm-up). PSUM accumulation is f32. Engine assignment: PE matmuls,
ACT exp only, DVE bias-adds/normalize, GPSIMD all psum->sbuf casts + W/x DMAs.
"""

import numpy as np

_CACHE: dict = {}

P = 128
BL, N, D, H, HD = 2, 1024, 768, 12, 64
D3 = 3 * D
SCALE = float(HD) ** -0.5


def _build():
    import concourse.mybir as mybir
    import concourse.tile as tile
    from concourse import bacc

    dt = mybir.dt
    F32, F16 = dt.float32, dt.float16
    AF = mybir.ActivationFunctionType

    nc = bacc.Bacc("TRN2", target_bir_lowering=False, debug=False)
    x_d = nc.dram_tensor("x", [BL, N, D], F32, kind="ExternalInput").ap()
    wqkv_d = nc.dram_tensor("W_qkv", [D, D3], F32, kind="ExternalInput").ap()
    bqkv_d = nc.dram_tensor("b_qkv", [D3], F32, kind="ExternalInput").ap()
    wproj_d = nc.dram_tensor("W_proj", [D, D], F32, kind="ExternalInput").ap()
    bproj_d = nc.dram_tensor("b_proj", [D], F32, kind="ExternalInput").ap()
    out_d = nc.dram_tensor("out", [BL, N, D], F32, kind="ExternalOutput").ap()
    # token-interleaved views: partition p, slot i <-> token 8p+i
    x_il = x_d.rearrange("b (p i) d -> b p (i d)", p=P)       # [2, 128, 6144]
    out_il = out_d.rearrange("b (p i) d -> b i p d", p=P)     # [2, 8, 128, 768]

    with tile.TileContext(nc) as tc:
        with tc.tile_pool(name="sb", bufs=1) as sb, \
             tc.tile_pool(name="dp", bufs=1, space="DRAM") as dp, \
             tc.tile_pool(name="ps", bufs=2, space="PSUM") as ps:

            # ---------- constants ----------
            ones_h = sb.tile([P, P], F16, tag="ones_h", bufs=1, name="ones_h")
            nc.vector.memset(ones_h[:], 1.0)

            # ---------- x + W loads: HWDGE f32 stage + gpsimd casts -------
            # x on sync queue; W staged on scalar queue (ACT idle pre-attn)
            x6 = {}
            x_h = {}
            x_h[0] = sb.tile([P, 8 * D], F16, tag="x_h", bufs=1, name="x_h")
            for t in range(8):
                xst = sb.tile([P, D], F32, tag="x6", bufs=4, name="xst")
                eng = nc.sync if t % 2 == 0 else nc.scalar
                eng.dma_start(xst[:], x_il[0][:, D * t:D * (t + 1)])
                nc.vector.tensor_copy(x_h[0][:, D * t:D * (t + 1)], xst[:])
            # separate Q/K/V weight tiles; per-queue DMA bw is ~110GB/s so
            # loads are spread: Q,V on gpsimd SWDGE (casts in-flight),
            # x/K/wp split across sync+scalar HWDGE queues
            wQ = [sb.tile([P, D], F16, tag=f"wq{d}", bufs=1, name=f"wq{d}")
                  for d in range(6)]
            wK = [sb.tile([P, D], F16, tag=f"wk{d}", bufs=1, name=f"wk{d}")
                  for d in range(6)]
            wV = [sb.tile([P, D], F16, tag=f"wv{d}", bufs=1, name=f"wv{d}")
                  for d in range(6)]
            for d in range(6):
                nc.gpsimd.dma_start(wQ[d][:], wqkv_d[P * d:P * (d + 1), 0:D])
            for d in range(6):
                nc.gpsimd.dma_start(wV[d][:],
                                    wqkv_d[P * d:P * (d + 1), 2 * D:3 * D])
            for d in range(6):
                wstg = sb.tile([P, D], F32, tag="wstage", bufs=3, name="wstg")
                eng = nc.sync if d % 2 == 0 else nc.scalar
                eng.dma_start(wstg[:], wqkv_d[P * d:P * (d + 1), D:2 * D])
                nc.vector.tensor_copy(wK[d][:], wstg[:])

            # ---------- biases: pad-to-32 + XBAR transpose (sync q first)
            bstg = sb.tile([18, P], F32, tag="bstg", bufs=1, name="bstg")
            nc.sync.dma_start(bstg[0:18, :],
                              bqkv_d.rearrange("(j p) -> j p", p=P))
            bstg_h = sb.tile([32, P], F16, tag="bstg_h", bufs=1, name="bstg_h")
            nc.vector.memset(bstg_h[:], 0.0)
            nc.vector.tensor_copy(bstg_h[0:18, :], bstg[0:18, :])
            bqkvT_h = sb.tile([P, 32], F16, tag="bqkvT_h", bufs=1,
                              name="bqkvT_h")
            nc.sync.dma_start_transpose(bqkvT_h[:], bstg_h[:])
            bqkvT = sb.tile([P, 18], F32, tag="bqkvT", bufs=1, name="bqkvT")
            nc.vector.tensor_copy(bqkvT[:], bqkvT_h[:, 0:18])
            bv_h = bqkvT_h  # [:, 12:18] = V bias, f16, lhsT of bfinal matmul
            bprow_s = sb.tile([P, D], F32, tag="wstage", bufs=3, name="bprow_s")
            nc.scalar.dma_start(bprow_s[0:1, :], bproj_d.unsqueeze(0))
            bproj_row = sb.tile([1, D], F16, tag="bproj_row", bufs=1,
                                name="bproj_row")
            nc.vector.tensor_copy(bproj_row[:], bprow_s[0:1, :])

            # ---------- PE warm-up: ~4us of dummy matmuls flips HAM to 8/8
            warm_h = sb.tile([P, 512], F16, tag="e", bufs=3, name="warm_h")
            nc.vector.memset(warm_h[:], 0.0)
            for wi in range(22):
                wps = ps.tile([P, 512], F32, tag="mm", bufs=2, name="wps")
                nc.tensor.matmul(wps[:], ones_h[:, 0:P], warm_h[:],
                                 start=True, stop=True)

            # ---------- x transposes: XBAR DMA-transpose, no PE ----------
            # xT_b[b][dp, j, n] = x token (8c+i at n=128i+c), dim 128j+dp
            xT_b = {b: sb.tile([P, 6, N], F16, tag=f"xT{b}", bufs=1,
                               name=f"xT{b}") for b in range(BL)}
            for t in range(8):
                nc.sync.dma_start_transpose(
                    xT_b[0][:, :, P * t:P * (t + 1)],
                    x_h[0][:, D * t:D * (t + 1)])
            # x6/x_h buffers shared (bufs=1): b1 load after b0 readers
            x_h[1] = sb.tile([P, 8 * D], F16, tag="x_h", bufs=1, name="x_h")
            for t in range(8):
                xst = sb.tile([P, D], F32, tag="x6", bufs=4, name="xst")
                eng = nc.sync if t % 2 == 0 else nc.scalar
                eng.dma_start(xst[:], x_il[1][:, D * t:D * (t + 1)])
                nc.vector.tensor_copy(x_h[1][:, D * t:D * (t + 1)], xst[:])
            wp_h = [sb.tile([P, D], F16, tag=f"wproj{d}", bufs=1,
                            name=f"wproj{d}") for d in range(6)]
            for d in range(6):
                wstg = sb.tile([P, D], F32, tag="wstage", bufs=3, name="wstg")
                nc.scalar.dma_start(wstg[:], wproj_d[P * d:P * (d + 1), :])
                nc.vector.tensor_copy(wp_h[d][:], wstg[:])
            for t in range(8):
                nc.sync.dma_start_transpose(
                    xT_b[1][:, :, P * t:P * (t + 1)],
                    x_h[1][:, D * t:D * (t + 1)])

            # ---------- per-batch tiles ----------
            qk = {b: [sb.tile([P, N], F16, tag=f"qk{b % 2}_{j}", bufs=1,
                              name=f"qk{j}") for j in range(12)]
                  for b in range(BL)}
            v = {b: [sb.tile([P, 12 * 65], F16, tag=f"v{b % 2}_{t}", bufs=1,
                             name=f"v{t}") for t in range(8)]
                 for b in range(BL)}
            at = {b: [sb.tile([P, N], F16, tag=f"at{j}", bufs=1,
                              name=f"at{j}") for j in range(6)]
                  for b in range(BL)}

            def emit_qkv_group(b, j, nh):
                qps = ps.tile([P, 512], F32, tag="mm", bufs=2, name="qps")
                w, col = (wQ, j) if j < 6 else (wK, j - 6)
                for d in range(6):
                    nc.tensor.matmul(qps[:], w[d][:, P * col:P * (col + 1)],
                                     xT_b[b][:, d, 512 * nh:512 * (nh + 1)],
                                     start=(d == 0), stop=(d == 5))
                dst = qk[b][j][:, 512 * nh:512 * (nh + 1)]
                if j < 6:   # Q: add bias on DVE
                    nc.vector.tensor_scalar_add(dst, qps[:], bqkvT[:, j:j + 1])
                else:       # K: bias dropped (softmax-invariant); plain cast
                    nc.vector.tensor_copy(dst, qps[:])

            def emit_v_group(b, t, ci):
                c0, cw = ((0, 512), (512, 256))[ci]
                v3 = v[b][t].rearrange("p (h c) -> p h c", c=65)
                if ci == 0:
                    nc.vector.tensor_copy(v3[:, :, 64:65],
                                          ones_h[:, 0:12].unsqueeze(2))
                vps = ps.tile([P, 512], F32, tag="mm", bufs=2, name="vps")
                for d in range(6):
                    nc.tensor.matmul(vps[:, 0:cw],
                                     xT_b[b][:, d, P * t:P * (t + 1)],
                                     wV[d][:, c0:c0 + cw],
                                     start=(d == 0), stop=(d == 5))
                nc.vector.tensor_copy(
                    v3[:, (c0 // HD):((c0 + cw) // HD), 0:HD],
                    vps[:, 0:cw].rearrange("p (h c) -> p h c", c=HD))

            def emit_proj_half(b, t, ci):
                c0, cw = ((0, 512), (512, 256))[ci]
                pps = ps.tile([P, 512], F32, tag="mm", bufs=2, name="pps")
                for d in range(6):
                    nc.tensor.matmul(pps[:, 0:cw],
                                     at[b][d][:, P * t:P * (t + 1)],
                                     wp_h[d][:, c0:c0 + cw],
                                     start=(d == 0), stop=False)
                nc.tensor.matmul(pps[:, 0:cw], ones_h[0:1, 0:P],
                                 bfinal_h[:, c0:c0 + cw],
                                 start=False, stop=True)
                osb = sb.tile([P, 512], F32, tag="outs", bufs=1, name="osb")
                if b == 0:  # fillers inside attn-1: ACT is exp-saturated
                    nc.vector.tensor_copy(osb[:, 0:cw], pps[:, 0:cw])
                else:       # tail: ACT idle
                    nc.scalar.copy(osb[:, 0:cw], pps[:, 0:cw])
                nc.sync.dma_start(out_il[b, t][:, c0:c0 + cw], osb[:, 0:cw])

            def emit_proj_group(b, t):
                emit_proj_half(b, t, 0)
                emit_proj_half(b, t, 1)

            # b_final = W_proj^T b_v + b_proj   [1, 768] f16
            bfinal_h = sb.tile([1, D], F16, tag="bfinal", bufs=1, name="bfinal")

            def emit_bfinal():
                for c0, cw in ((0, 512), (512, 256)):
                    bf_ps = ps.tile([1, 512], F32, tag="mm", bufs=2,
                                    name="bf_ps")
                    for d in range(6):
                        nc.tensor.matmul(bf_ps[:, 0:cw],
                                         bv_h[:, 12 + d:13 + d],
                                         wp_h[d][:, c0:c0 + cw],
                                         start=(d == 0), stop=(d == 5))
                    nc.vector.tensor_add(bfinal_h[:, c0:c0 + cw],
                                         bf_ps[0:1, 0:cw],
                                         bproj_row[:, c0:c0 + cw])

            def emit_attention(b, fillers):
                for jp in range(6):
                    qt, kt = qk[b][jp], qk[b][6 + jp]
                    for nh in range(2):
                        n0 = 512 * nh
                        ot = [ps.tile([65, 512], F32, tag="ot", bufs=2,
                                      name="otps") for _ in range(2)]
                        pend = []
                        for m in range(8):
                            sps = ps.tile([P, N], F32, tag="s", bufs=2,
                                          name="sps")
                            for hh in range(2):
                                r0, r1 = HD * hh, HD * (hh + 1)
                                nc.tensor.matmul(
                                    sps[:, 512 * hh:512 * (hh + 1)],
                                    kt[r0:r1, P * m:P * (m + 1)],
                                    qt[r0:r1, n0:n0 + 512],
                                    start=True, stop=True)
                            e = sb.tile([P, N], F16, tag="e", bufs=3, name="e")
                            nc.scalar.activation(e[:], sps[:], AF.Exp,
                                                 scale=SCALE)
                            pend.append((m, e))
                            if len(pend) == 2:
                                pm, pe_ = pend.pop(0)
                                for hh in range(2):
                                    h = 2 * jp + hh
                                    nc.tensor.matmul(
                                        ot[hh][:],
                                        v[b][pm][:, 65 * h:65 * h + 65],
                                        pe_[:, 512 * hh:512 * (hh + 1)],
                                        start=(pm == 0), stop=(pm == 7))
                            if m in (1, 3, 5) and fillers:
                                fillers.pop(0)()
                        for pm, pe_ in pend:
                            for hh in range(2):
                                h = 2 * jp + hh
                                nc.tensor.matmul(
                                    ot[hh][:], v[b][pm][:, 65 * h:65 * h + 65],
                                    pe_[:, 512 * hh:512 * (hh + 1)],
                                    start=(pm == 0), stop=(pm == 7))
                        # normalize: copy O^T + denom out of PSUM early (frees
                        # ot banks), then batched recip/broadcast off-chain
                        u_sb = [sb.tile([HD, 512], F16, tag=f"u{hh}", bufs=1,
                                        name="u_sb") for hh in range(2)]
                        den = sb.tile([1, N], F32, tag="den", bufs=1,
                                      name="den")
                        nc.vector.tensor_copy(u_sb[0][:], ot[0][0:HD, :])
                        nc.vector.tensor_copy(den[:, 0:512], ot[0][64:65, :])
                        nc.vector.tensor_copy(u_sb[1][:], ot[1][0:HD, :])
                        nc.vector.tensor_copy(den[:, 512:1024],
                                              ot[1][64:65, :])
                        rr_f = sb.tile([1, N], F32, tag="rr_f", bufs=1,
                                       name="rr_f")
                        nc.vector.reciprocal_approx_fast(out=rr_f[:],
                                                         in_=den[:])
                        rr_h = sb.tile([1, N], F16, tag="rr_h", bufs=1,
                                       name="rr_h")
                        nc.vector.tensor_copy(rr_h[:], rr_f[:])
                        bc2 = sb.tile([HD, N], F16, tag="bc2", bufs=2,
                                      name="bc2")
                        nc.gpsimd.partition_broadcast(bc2[:], rr_h[:],
                                                      channels=HD)
                        for hh in range(2):
                            r0, r1 = HD * hh, HD * (hh + 1)
                            nc.vector.tensor_mul(
                                at[b][jp][r0:r1, n0:n0 + 512],
                                u_sb[hh][:],
                                bc2[:, 512 * hh:512 * (hh + 1)])
                        if fillers:
                            fillers.pop(0)()

            # ---------- schedule ----------
            # batch 0 QKV + V fully before attention (wire order matches);
            # PE stays continuously busy so HAM holds 8/8
            for j in range(12):
                for nh in range(2):
                    emit_qkv_group(0, j, nh)
            for t in range(8):
                for ci in range(2):
                    emit_v_group(0, t, ci)

            fill0 = [lambda j=j, nh=nh: emit_qkv_group(1, j, nh)
                     for jp_ in range(6) for j in (jp_, 6 + jp_)
                     for nh in range(2)]
            fill0 += [lambda t=t, ci=ci: emit_v_group(1, t, ci)
                      for t in range(8) for ci in range(2)]
            emit_attention(0, fill0)
            for f in fill0:
                f()

            # batch-1 attention with bfinal + batch-0 projection interleaved
            fill1 = [emit_bfinal]
            fill1 += [lambda t=t, ci=ci: emit_proj_half(0, t, ci)
                      for t in range(8) for ci in range(2)]
            emit_attention(1, fill1)
            for f in fill1:
                f()

            # batch-1 projection (tail)
            for t in range(8):
                emit_proj_group(1, t)
    nc.compile()
    return nc


def _get_nc():
    if "nc" not in _CACHE:
        _CACHE["nc"] = _build()
    return _CACHE["nc"]


def kernel(x, W_qkv, b_qkv, W_proj, b_proj):
    from concourse.bass_utils import run_bass_kernel_spmd

    nc = _get_nc()
    x = np.ascontiguousarray(x, dtype=np.float32)
    in_maps = [
        {
            "x": x[2 * i:2 * i + 2],
            "W_qkv": np.asarray(W_qkv, dtype=np.float32),
            "b_qkv": np.asarray(b_qkv, dtype=np.float32),
            "W_proj": np.asarray(W_proj, dtype=np.float32),
            "b_proj": np.asarray(b_proj, dtype=np.float32),
        }
        for i in range(8)
    ]
    res = run_bass_kernel_spmd(nc, in_maps, core_ids=list(range(8)))
    return np.concatenate([r["out"] for r in res.results], axis=0)
